# revision 1
# baseline (speedup 1.0000x reference)
"""Trainium2 Bass kernel for nn_Graph_Enhance_model (GNN message passing).

Self-contained: hardcodes shapes B=4,F=32,H=8,O=16,D=2048, 8 cores.
Data-parallel over the 128 (b,f) frames: 16 frames per core.
"""

import os
import sys

for _p in ("/opt/trn_rl_repo", "/opt/pypackages"):
    if _p not in sys.path and os.path.isdir(_p):
        sys.path.append(_p)

import numpy as np
import ml_dtypes

import concourse.bass as bass
import concourse.bacc as bacc
import concourse.tile as tile
import concourse.mybir as mybir
from concourse import bass_utils
from concourse.masks import make_identity

BF16 = mybir.dt.bfloat16
F32 = mybir.dt.float32
AF = mybir.ActivationFunctionType
ALU = mybir.AluOpType
AX = mybir.AxisListType

NB = ml_dtypes.bfloat16

B, F, H, O, D = 4, 32, 8, 16, 2048
NFRAMES = B * F          # 128
NCORES = 8
FPC = NFRAMES // NCORES  # 16 frames per core
ROWS = H * O             # 128 rows per frame
KC = D // 128            # 16 K-chunks
NQ = FPC // 4            # 4 quads of 4 frames

_CACHE = {}




def _combine_e(nc, step, mt, q, pe, wb_sb, bet_sb, um1t, msum_f, pool):
    """UM = (msg_e_psum + be) * w ; step 1 also reduces over o into msum."""
    if step == 0:
        nc.vector.scalar_tensor_tensor(out=um1t[:, mt, :], in0=pe,
                                       scalar=bet_sb[:, mt:mt + 1], in1=wb_sb,
                                       op0=ALU.add, op1=ALU.mult)
    else:
        tmp = pool.tile([128, 512], F32, tag="um2")
        nc.vector.scalar_tensor_tensor(out=tmp, in0=pe,
                                       scalar=bet_sb[:, mt:mt + 1], in1=wb_sb,
                                       op0=ALU.add, op1=ALU.mult)
        nc.vector.reduce_sum(msum_f[:, mt, q * 32:(q + 1) * 32],
                             tmp.rearrange("p (f h o) -> p f h o", f=4, h=8),
                             axis=AX.X)


def _build_nc():
    nc = bacc.Bacc("TRN2", target_bir_lowering=False, debug=False, num_devices=NCORES)

    dt_in = {}

    def din(name, shape, dt):
        dt_in[name] = nc.dram_tensor(name, shape, dt, kind="ExternalInput")
        return dt_in[name]

    e0t = din("e0t", [NQ, D, 512], BF16)
    ot = din("ot", [D, FPC * O], BF16)
    ht_bd = din("ht_b", [D, FPC * H], BF16)
    h_rmd = din("h_rm", [FPC * H, D], F32)
    pmatd = din("pmat", [FPC * H, FPC], BF16)
    scsf = din("scsf", [D, 3 * FPC], BF16)
    sc4rm = din("sc4rm", [FPC, D], F32)
    sfrm = din("sfrm", [FPC, D], F32)
    wcat = din("wcat", [D, D], BF16)
    bl1td = din("bl1t", [128, 8], F32)
    betd = din("bet", [128, 8], F32)
    wnt = din("wnt", [D, D // 2], BF16)
    wnb = din("wnb", [1, D // 2], BF16)
    wl2 = din("wl2", [128, 8], BF16)
    ghi = din("ghi", [D, 3 * D], BF16)
    ghib = din("ghib", [1, 3 * D], BF16)
    ghh = din("ghh", [D, 3 * D], BF16)
    ghhb = din("ghhb", [1, 3 * D], BF16)
    gsi = din("gsi", [D, 3 * D], BF16)
    gsib = din("gsib", [1, 3 * D], BF16)
    gsh = din("gsh", [D, 3 * D], BF16)
    gshb = din("gshb", [1, 3 * D], BF16)
    outp = nc.dram_tensor("outp", [FPC, D], F32, kind="ExternalOutput")

    from contextlib import ExitStack

    with tile.TileContext(nc) as tc, ExitStack() as ctx:
        glob = ctx.enter_context(tc.tile_pool(name="glob", bufs=1))

        ones_b = glob.tile([1, 512], BF16)
        nc.vector.memset(ones_b, 1.0)
        ones16 = glob.tile([1, 16], BF16)
        nc.vector.memset(ones16, 1.0)
        ident16 = glob.tile([16, 16], BF16)
        make_identity(nc, ident16)

        wl2_sb = glob.tile([128, 8], BF16)
        nc.sync.dma_start(out=wl2_sb, in_=wl2.ap())
        bl1t_sb = glob.tile([128, 8], F32)
        nc.sync.dma_start(out=bl1t_sb, in_=bl1td.ap())
        bet_sb = glob.tile([128, 8], F32)
        nc.sync.dma_start(out=bet_sb, in_=betd.ap())

        msgn_sb = glob.tile([128, 8, FPC * O], BF16)    # [1024, 256] transposed msg_n
        msum_f = glob.tile([128, KC, FPC * H], F32)     # M_sum2^T (raw sum over o)
        msum_b = glob.tile([128, KC, FPC * H], BF16)
        ah_pad = glob.tile([128, KC, 3 * FPC], BF16)    # [All_human^T/8 | zeros]
        nc.vector.memset(ah_pad, 0.0)
        ht_b = glob.tile([128, KC, FPC * H], BF16)
        scsf_b = glob.tile([128, KC, 3 * FPC], BF16)    # [S_C4^T | 0 | S_f^T]

        nc.sync.dma_start(out=ht_b, in_=ht_bd.ap().rearrange("(kc p) n -> p kc n", p=128))
        nc.sync.dma_start(out=scsf_b, in_=scsf.ap().rearrange("(kc p) n -> p kc n", p=128))

        with (
            tc.tile_pool(name="pwcat", bufs=1) as pwcat,
            tc.tile_pool(name="pa", bufs=2) as pa,
            tc.tile_pool(name="pa1", bufs=1) as pa1,
        ):
            wcat_sb = pwcat.tile([128, KC, D], BF16)
            nc.sync.dma_start(out=wcat_sb, in_=wcat.ap().rearrange("(kc p) m -> p kc m", p=128))

            # ---------------- Phase 0: msg_n^T = Wn @ O^T + bn ----------------
            with (
                tc.tile_pool(name="p0", bufs=1) as p0,
                tc.tile_pool(name="p0ps", bufs=4, space="PSUM") as p0ps,
            ):
                wnb_sb = p0.tile([1, D // 2], BF16)
                nc.sync.dma_start(out=wnb_sb, in_=wnb.ap())
                ot_sb = p0.tile([128, KC, FPC * O], BF16)
                nc.sync.dma_start(out=ot_sb, in_=ot.ap().rearrange("(kc p) n -> p kc n", p=128))
                for half in range(2):
                    wn_sb = p0.tile([128, KC, 512], BF16, tag="wn")
                    nc.sync.dma_start(out=wn_sb, in_=wnt.ap()[:, half * 512:(half + 1) * 512]
                                      .rearrange("(kc p) m -> p kc m", p=128))
                    for mt4 in range(4):
                        mt = half * 4 + mt4
                        pm = p0ps.tile([128, FPC * O], F32, tag="pm")
                        for kc in range(KC):
                            nc.tensor.matmul(pm, lhsT=wn_sb[:, kc, mt4 * 128:(mt4 + 1) * 128],
                                             rhs=ot_sb[:, kc, :], start=(kc == 0), stop=False)
                        nc.tensor.matmul(pm, lhsT=wnb_sb[0:1, mt * 128:(mt + 1) * 128],
                                         rhs=ones_b[0:1, 0:FPC * O], start=False, stop=True)
                        nc.scalar.copy(msgn_sb[:, mt, :], pm)

        # ---------------- Phase A: 2 propagation steps over edges ----------------
            with tc.tile_pool(name="paps", bufs=4, space="PSUM") as paps, \
                 tc.tile_pool(name="papss", bufs=2, space="PSUM") as papss:
                for q in range(NQ):
                    xq = pa.tile([128, KC, 512], BF16, tag="xq")
                    nc.sync.dma_start(out=xq, in_=e0t.ap()[q].rearrange("(kc p) n -> p kc n", p=128))
                    um1t = pa1.tile([128, KC, 512], BF16, tag="um1t")
                    for step in range(2):
                        rhs = xq if step == 0 else um1t
                        # --- a-wave: relu(X @ Wl1^T + bl1), transposed ---
                        relu_sb = pa1.tile([128, 8, 512], BF16, tag="relu")
                        for mt in range(8, 16):
                            pw_a = paps.tile([128, 512], F32, tag="wave")
                            for kc in range(KC):
                                nc.tensor.matmul(pw_a, lhsT=wcat_sb[:, kc, mt * 128:(mt + 1) * 128],
                                                 rhs=rhs[:, kc, :], start=(kc == 0), stop=(kc == KC - 1))
                            nc.scalar.activation(relu_sb[:, mt - 8, :], pw_a, AF.Relu,
                                                 bias=bl1t_sb[:, mt - 8:mt - 7])
                        # --- logits + softmax over o (groups of 16) ---
                        pl = papss.tile([1, 512], F32, tag="pl")
                        for kc2 in range(8):
                            nc.tensor.matmul(pl, lhsT=wl2_sb[:, kc2:kc2 + 1],
                                             rhs=relu_sb[:, kc2, :], start=(kc2 == 0), stop=(kc2 == 7))
                        pl3 = pl.rearrange("o (g i) -> o g i", i=16)
                        mx = pa1.tile([1, 32], F32, tag="mx")
                        nc.vector.reduce_max(mx, pl3, axis=AX.X)
                        sub = pa1.tile([1, 512], F32, tag="sub")
                        nc.vector.tensor_tensor(sub.rearrange("o (g i) -> o g i", i=16), pl3,
                                                mx.broadcast_to((1, 32, 16)), op=ALU.subtract)
                        nc.scalar.activation(sub, sub, AF.Exp)
                        ex3 = sub.rearrange("o (g i) -> o g i", i=16)
                        sm = pa1.tile([1, 32], F32, tag="sm")
                        nc.vector.reduce_sum(sm, ex3, axis=AX.X)
                        rs = pa1.tile([1, 32], F32, tag="rs")
                        nc.vector.reciprocal(rs, sm)
                        w_sb = pa1.tile([1, 512], BF16, tag="w")
                        nc.vector.tensor_tensor(w_sb.rearrange("o (g i) -> o g i", i=16), ex3,
                                                rs.broadcast_to((1, 32, 16)), op=ALU.mult)
                        # --- msg_e wave; w-broadcast MM emitted after 2 groups ---
                        e_ps = []
                        wb_sb = pa1.tile([128, 512], F32, tag="wb")
                        for mt in range(8):
                            pe = paps.tile([128, 512], F32, tag="wave")
                            for kc in range(KC):
                                nc.tensor.matmul(pe, lhsT=wcat_sb[:, kc, mt * 128:(mt + 1) * 128],
                                                 rhs=rhs[:, kc, :], start=(kc == 0), stop=(kc == KC - 1))
                            e_ps.append(pe)
                            if mt == 1:
                                # broadcast w along partitions via K=1 matmul (PE waits
                                # here on softmax, hidden under the first 2 MM groups)
                                pw_b = papss.tile([128, 512], F32, tag="pw")
                                nc.tensor.matmul(pw_b, lhsT=ones_b[0:1, 0:128], rhs=w_sb,
                                                 start=True, stop=True)
                                nc.scalar.copy(wb_sb, pw_b)
                            if mt >= 1:
                                for cmt in ([0, 1] if mt == 1 else [mt]):
                                    _combine_e(nc, step, cmt, q, e_ps[cmt], wb_sb, bet_sb,
                                               um1t, msum_f, pa1)
                        wb4 = wb_sb.rearrange("p (f h o) -> p f h o", f=4, h=8)
                        # msg_n half (tiles 8..16): broadcast over h
                        for j in range(8):
                            mt = 8 + j
                            base = msgn_sb[:, j, q * 64:(q + 1) * 64]
                            mn_bc = bass.AP(tensor=base.tensor, offset=base.offset,
                                            ap=[list(base.ap[0]), [16, 4], [0, 8], [1, 16]])
                            if step == 0:
                                nc.vector.tensor_tensor(
                                    um1t[:, mt, :].rearrange("p (f h o) -> p f h o", f=4, h=8),
                                    mn_bc, wb4, op=ALU.mult)
                            else:
                                tmp = pa1.tile([128, 512], F32, tag="um2")
                                nc.vector.tensor_tensor(
                                    tmp.rearrange("p (f h o) -> p f h o", f=4, h=8),
                                    mn_bc, wb4, op=ALU.mult)
                                nc.vector.reduce_sum(msum_f[:, mt, q * 32:(q + 1) * 32],
                                                     tmp.rearrange("p (f h o) -> p f h o", f=4, h=8),
                                                     axis=AX.X)


            for kc in range(KC):
                nc.vector.tensor_copy(msum_b[:, kc, :], msum_f[:, kc, :])

        # ---------------- Phase B: human GRU (row-major, weights moving) ----------------
        with (
            tc.tile_pool(name="pb", bufs=2) as pb,
            tc.tile_pool(name="pb1", bufs=1) as pb1,
            tc.tile_pool(name="pbps", bufs=1, space="PSUM") as pbps,
            tc.tile_pool(name="pbps2", bufs=2, space="PSUM") as pbps2,
        ):
            NR = FPC * H  # 128 rows
            h_rm = pb1.tile([NR, D], F32)
            nc.sync.dma_start(out=h_rm, in_=h_rmd.ap())
            pmat_sb = pb1.tile([NR, FPC], BF16)
            nc.sync.dma_start(out=pmat_sb, in_=pmatd.ap())
            hum_b = pb1.tile([NR, D], BF16)

            def gh_block(j, pt, use_i, use_h, lastfix=None):
                """accumulate gi (wih@msum) and/or gh (whh@ht) for gate block j
                into psum pt, row-major [128 rows, 512 gates]."""
                ops = []
                if use_h:
                    wb_t = pb.tile([128, KC, 512], BF16, tag="bwh")
                    nc.sync.dma_start(out=wb_t, in_=ghh.ap()[:, j * 512:(j + 1) * 512]
                                      .rearrange("(kc p) m -> p kc m", p=128))
                    bb = pb.tile([1, 512], BF16, tag="bbh")
                    nc.sync.dma_start(out=bb, in_=ghhb.ap()[:, j * 512:(j + 1) * 512])
                    ops += [(wb_t, ht_b, kc) for kc in range(KC)] + [(bb, None, None)]
                if use_i:
                    wi_t = pb.tile([128, KC, 512], BF16, tag="bwi")
                    nc.sync.dma_start(out=wi_t, in_=ghi.ap()[:, j * 512:(j + 1) * 512]
                                      .rearrange("(kc p) m -> p kc m", p=128))
                    bi = pb.tile([1, 512], BF16, tag="bbi")
                    nc.sync.dma_start(out=bi, in_=ghib.ap()[:, j * 512:(j + 1) * 512])
                    ops += [(wi_t, msum_b, kc) for kc in range(KC)] + [(bi, None, None)]
                for idx, (w, x, kc) in enumerate(ops):
                    st, sp = idx == 0, idx == len(ops) - 1
                    if x is None:
                        nc.tensor.matmul(pt, lhsT=ones_b[0:1, 0:128], rhs=w[0:1, :],
                                         start=st, stop=sp)
                    else:
                        nc.tensor.matmul(pt, lhsT=x[:, kc, :], rhs=w[:, kc, :],
                                         start=st, stop=sp)

            for t in range(4):
                cols = slice(t * 512, (t + 1) * 512)
                p_r = pbps.tile([NR, 512], F32, tag="pr")
                gh_block(t, p_r, True, True)
                p_z = pbps.tile([NR, 512], F32, tag="pz")
                gh_block(4 + t, p_z, True, True)
                p_in = pbps.tile([NR, 512], F32, tag="pin")
                gh_block(8 + t, p_in, True, False)
                p_hn = pbps.tile([NR, 512], F32, tag="phn")
                gh_block(8 + t, p_hn, False, True)
                r_sb = pb1.tile([NR, 512], F32, tag="r")
                nc.scalar.activation(r_sb, p_r, AF.Sigmoid)
                z_sb = pb1.tile([NR, 512], F32, tag="z")
                nc.scalar.activation(z_sb, p_z, AF.Sigmoid)
                t1 = pb1.tile([NR, 512], F32, tag="t1")
                nc.vector.tensor_tensor(t1, r_sb, p_hn, op=ALU.mult)
                t2 = pb1.tile([NR, 512], F32, tag="t2")
                nc.vector.tensor_tensor(t2, t1, p_in, op=ALU.add)
                n_sb = pb1.tile([NR, 512], F32, tag="n")
                nc.scalar.activation(n_sb, t2, AF.Tanh)
                t3 = pb1.tile([NR, 512], F32, tag="t3")
                nc.vector.tensor_tensor(t3, h_rm[:, cols], n_sb, op=ALU.subtract)
                t4 = pb1.tile([NR, 512], F32, tag="t4")
                nc.vector.tensor_tensor(t4, z_sb, t3, op=ALU.mult)
                nc.vector.tensor_tensor(hum_b[:, cols], n_sb, t4, op=ALU.add)
            # All_human^T chunks via PE: ah[c] = hum[:, c-chunk].T @ pmat
            for c in range(KC):
                pah = pbps2.tile([128, FPC], F32, tag="pah")
                nc.tensor.matmul(pah, lhsT=hum_b[:, c * 128:(c + 1) * 128], rhs=pmat_sb,
                                 start=True, stop=True)
                nc.scalar.copy(ah_pad[:, c, 0:FPC], pah)


        # ---------------- Phase C: two S-node GRUs (stacked M=32 stationaries) ----------------
        with (
            tc.tile_pool(name="pc", bufs=2) as pc,
            tc.tile_pool(name="pc1", bufs=1) as pc1,
            tc.tile_pool(name="pcsm", bufs=1) as pcsm,
            tc.tile_pool(name="pcps", bufs=2, space="PSUM") as pcps,
            tc.tile_pool(name="pctps", bufs=2, space="PSUM") as pctps,
        ):
            sc4rm_sb = pc1.tile([FPC, D], F32)
            nc.sync.dma_start(out=sc4rm_sb, in_=sc4rm.ap())
            sfrm32 = pc1.tile([3 * FPC, D], F32)
            nc.sync.dma_start(out=sfrm32[2 * FPC:3 * FPC, :], in_=sfrm.ap())
            g1_sb = pc1.tile([16, 12, 512], BF16)    # gi1+gh1 (r,z) / inn (n-blocks)
            gh1n_sb = pc1.tile([16, 4, 512], BF16)   # hn1
            gh2_sb = pc1.tile([48, 12, 512], BF16)   # whh @ sf + bhh (rows 32:48)
            g2i_sb = pc1.tile([48, 12, 512], BF16)
            s1_sb = pc1.tile([16, D], BF16)
            s1t_pad = pc1.tile([128, KC, 3 * FPC], BF16)   # [zeros | s1^T]
            nc.vector.memset(s1t_pad, 0.0)
            out32 = pc1.tile([3 * FPC, D], F32)

            for j in range(12):
                wsi = pc.tile([128, KC, 512], BF16, tag="wsi")
                nc.sync.dma_start(out=wsi, in_=gsi.ap()[:, j * 512:(j + 1) * 512].rearrange("(kc p) m -> p kc m", p=128))
                wsh = pc.tile([128, KC, 512], BF16, tag="wsh")
                nc.sync.dma_start(out=wsh, in_=gsh.ap()[:, j * 512:(j + 1) * 512].rearrange("(kc p) m -> p kc m", p=128))
                bsi = pc.tile([1, 512], BF16, tag="bsi")
                nc.sync.dma_start(out=bsi, in_=gsib.ap()[:, j * 512:(j + 1) * 512])
                bsh = pc.tile([1, 512], BF16, tag="bsh")
                nc.sync.dma_start(out=bsh, in_=gshb.ap()[:, j * 512:(j + 1) * 512])
                # PA rows 0:16 = gi1 (+bsi); rows 16:32 = zeros
                PA = pcps.tile([48, 512], F32, tag="PA")
                for kc in range(KC):
                    nc.tensor.matmul(PA, lhsT=ah_pad[:, kc, :], rhs=wsi[:, kc, :],
                                     start=(kc == 0), stop=False)
                nc.tensor.matmul(PA[0:16, :], lhsT=ones16, rhs=bsi[0:1, :], start=False, stop=(j >= 8))
                if j < 8:
                    # accumulate: rows 0:16 += gh1, rows 16:32 += gh2; +bsh on all
                    for kc in range(KC):
                        nc.tensor.matmul(PA, lhsT=scsf_b[:, kc, :], rhs=wsh[:, kc, :],
                                         start=False, stop=False)
                    nc.tensor.matmul(PA, lhsT=ones_b[0:1, 0:48], rhs=bsh[0:1, :],
                                     start=False, stop=True)
                    nc.scalar.copy(g1_sb[:, j, :], PA[0:16, :])
                    nc.scalar.copy(gh2_sb[32:48, j, :], PA[32:48, :])
                else:
                    nc.scalar.copy(g1_sb[:, j, :], PA[0:16, :])
                    PH = pcps.tile([48, 512], F32, tag="PH")
                    for kc in range(KC):
                        nc.tensor.matmul(PH, lhsT=scsf_b[:, kc, :], rhs=wsh[:, kc, :],
                                         start=(kc == 0), stop=False)
                    nc.tensor.matmul(PH, lhsT=ones_b[0:1, 0:48], rhs=bsh[0:1, :],
                                     start=False, stop=True)
                    nc.scalar.copy(gh1n_sb[:, j - 8, :], PH[0:16, :])
                    nc.scalar.copy(gh2_sb[32:48, j, :], PH[32:48, :])

            # step-1 elementwise -> s1 (rows 0:16)
            for t in range(4):
                cols = slice(t * 512, (t + 1) * 512)
                z1 = pcsm.tile([16, 512], F32, tag="z1")
                nc.scalar.activation(z1, g1_sb[:, 4 + t, :], AF.Sigmoid)
                r1 = pcsm.tile([16, 512], F32, tag="r1")
                nc.scalar.activation(r1, g1_sb[:, t, :], AF.Sigmoid)
                u1 = pcsm.tile([16, 512], F32, tag="u1")
                nc.vector.tensor_tensor(u1, r1, gh1n_sb[:, t, :], op=ALU.mult)
                u2 = pcsm.tile([16, 512], F32, tag="u2")
                nc.vector.tensor_tensor(u2, u1, g1_sb[:, 8 + t, :], op=ALU.add)
                n1 = pcsm.tile([16, 512], F32, tag="n1")
                nc.scalar.activation(n1, u2, AF.Tanh)
                u3 = pcsm.tile([16, 512], F32, tag="u3")
                nc.vector.tensor_tensor(u3, sc4rm_sb[:, cols], n1, op=ALU.subtract)
                u4 = pcsm.tile([16, 512], F32, tag="u4")
                nc.vector.tensor_tensor(u4, z1, u3, op=ALU.mult)
                nc.vector.tensor_tensor(s1_sb[:, cols], n1, u4, op=ALU.add)
            # transpose s1 -> s1t_pad cols 16:32 (bf16)
            for c in range(KC):
                ptp = pctps.tile([128, 16], BF16, tag="tp")
                nc.tensor.transpose(ptp, s1_sb[:, c * 128:(c + 1) * 128], ident16)
                nc.scalar.copy(s1t_pad[:, c, 2 * FPC:3 * FPC], ptp)
            # step 2: gi2 = wih @ s1 (+bih), rows 16:32
            for j in range(12):
                wsi = pc.tile([128, KC, 512], BF16, tag="wsi")
                nc.sync.dma_start(out=wsi, in_=gsi.ap()[:, j * 512:(j + 1) * 512].rearrange("(kc p) m -> p kc m", p=128))
                bsi = pc.tile([1, 512], BF16, tag="bsi")
                nc.sync.dma_start(out=bsi, in_=gsib.ap()[:, j * 512:(j + 1) * 512])
                PZ = pcps.tile([48, 512], F32, tag="PA")
                for kc in range(KC):
                    nc.tensor.matmul(PZ, lhsT=s1t_pad[:, kc, :], rhs=wsi[:, kc, :],
                                     start=(kc == 0), stop=False)
                nc.tensor.matmul(PZ, lhsT=ones_b[0:1, 0:48], rhs=bsi[0:1, :],
                                 start=False, stop=True)
                if j < 8:
                    nc.vector.tensor_tensor(g2i_sb[32:48, j, :], PZ[32:48, :],
                                            gh2_sb[32:48, j, :], op=ALU.add)
                else:
                    nc.scalar.copy(g2i_sb[32:48, j, :], PZ[32:48, :])
            # step-2 elementwise (rows 16:32) -> out
            for t in range(4):
                cols = slice(t * 512, (t + 1) * 512)
                z2 = pcsm.tile([48, 512], F32, tag="z2")
                nc.scalar.activation(z2[32:48, :], g2i_sb[32:48, 4 + t, :], AF.Sigmoid)
                r2 = pcsm.tile([48, 512], F32, tag="r2")
                nc.scalar.activation(r2[32:48, :], g2i_sb[32:48, t, :], AF.Sigmoid)
                v1 = pcsm.tile([48, 512], F32, tag="v1")
                nc.vector.tensor_tensor(v1[32:48, :], r2[32:48, :], gh2_sb[32:48, 8 + t, :], op=ALU.mult)
                v2 = pcsm.tile([48, 512], F32, tag="v2")
                nc.vector.tensor_tensor(v2[32:48, :], v1[32:48, :], g2i_sb[32:48, 8 + t, :], op=ALU.add)
                n2 = pcsm.tile([48, 512], F32, tag="n2")
                nc.scalar.activation(n2[32:48, :], v2[32:48, :], AF.Tanh)
                v3 = pcsm.tile([48, 512], F32, tag="v3")
                nc.vector.tensor_tensor(v3[32:48, :], sfrm32[32:48, cols], n2[32:48, :], op=ALU.subtract)
                v4 = pcsm.tile([48, 512], F32, tag="v4")
                nc.vector.tensor_tensor(v4[32:48, :], z2[32:48, :], v3[32:48, :], op=ALU.mult)
                nc.vector.tensor_tensor(out32[32:48, cols], n2[32:48, :], v4[32:48, :], op=ALU.add)
            nc.sync.dma_start(out=outp.ap(), in_=out32[32:48, :])

    nc.compile()
    return nc


def _prep_in_maps(inputs):
    E = np.ascontiguousarray(inputs["H_O_edges"].reshape(NFRAMES, ROWS, D))
    On = inputs["O_nodes"].reshape(NFRAMES, O, D)
    Hn = inputs["H_nodes"].reshape(NFRAMES, H, D)
    Sc4 = inputs["S_node_C4"].reshape(NFRAMES, D)
    Sf = np.ascontiguousarray(inputs["final_S_node"].transpose(0, 2, 1)).reshape(NFRAMES, D)

    shared = {
        "wcat": np.ascontiguousarray(
            np.concatenate([inputs["We"], inputs["Wl1"]], axis=0).T).astype(NB),
        "bl1t": np.ascontiguousarray(inputs["bl1"].reshape(8, 128).T).astype(np.float32),
        "bet": np.ascontiguousarray(inputs["be"].reshape(8, 128).T).astype(np.float32),
        "pmat": np.ascontiguousarray(np.kron(np.eye(FPC), np.ones((H, 1))) / H).astype(NB),
        "wnt": np.ascontiguousarray(inputs["Wn"].T).astype(NB),
        "wnb": inputs["bn"][None, :].astype(NB),
        "wl2": np.ascontiguousarray(inputs["Wl2"][0].reshape(8, 128).T).astype(NB),
        "ghi": np.ascontiguousarray((inputs["gh_wih"] / float(O)).T).astype(NB),
        "ghib": inputs["gh_bih"][None, :].astype(NB),
        "ghh": np.ascontiguousarray(inputs["gh_whh"].T).astype(NB),
        "ghhb": inputs["gh_bhh"][None, :].astype(NB),
        "gsi": np.ascontiguousarray(inputs["gs_wih"].T).astype(NB),
        "gsib": inputs["gs_bih"][None, :].astype(NB),
        "gsh": np.ascontiguousarray(inputs["gs_whh"].T).astype(NB),
        "gshb": inputs["gs_bhh"][None, :].astype(NB),
    }

    in_maps = []
    for c in range(NCORES):
        fr = slice(c * FPC, (c + 1) * FPC)
        Ec = E[fr]  # [16, 128, 2048]
        e0t = np.ascontiguousarray(
            Ec.reshape(NQ, 4, ROWS, D).transpose(0, 3, 1, 2).reshape(NQ, D, 512)).astype(NB)
        ot = np.ascontiguousarray(
            On[fr].reshape(FPC * O, D).T).astype(NB)
        ht = np.ascontiguousarray(Hn[fr].reshape(FPC * H, D).T)
        sc4 = Sc4[fr]
        sf = Sf[fr]
        m = dict(shared)
        m.update({
            "e0t": e0t,
            "ot": ot,
            "ht_b": ht.astype(NB),
            "h_rm": np.ascontiguousarray(Hn[fr].reshape(FPC * H, D)).astype(np.float32),
            "scsf": np.ascontiguousarray(np.concatenate(
                [sc4.T, np.zeros_like(sc4.T), sf.T], axis=1)).astype(NB),
            "sc4rm": np.ascontiguousarray(sc4).astype(np.float32),
            "sfrm": np.ascontiguousarray(sf).astype(np.float32),
        })
        in_maps.append(m)
    return in_maps


LAST_RESULT = None


def kernel(**inputs):
    global LAST_RESULT
    if "nc" not in _CACHE:
        _CACHE["nc"] = _build_nc()
    nc = _CACHE["nc"]
    in_maps = _prep_in_maps(inputs)
    trace = os.environ.get("KERNEL_TRACE", "0") == "1"
    res = bass_utils.run_bass_kernel_spmd(
        nc, in_maps, core_ids=list(range(NCORES)), trace=trace)
    LAST_RESULT = res
    out = np.concatenate([res.results[c]["outp"] for c in range(NCORES)], axis=0)
    return np.ascontiguousarray(out.reshape(B, F, D)).astype(np.float32)


if __name__ == "__main__":
    np.random.seed(0)
    ins = {
        "S_node_C4": np.random.randn(B, F, D).astype(np.float32),
        "final_S_node": np.random.randn(B, D, F).astype(np.float32),
        "H_nodes": np.random.randn(B, F, H, D).astype(np.float32),
        "O_nodes": np.random.randn(B, F, O, D).astype(np.float32),
        "H_O_edges": np.random.randn(B, F, H, O, D).astype(np.float32),
        "Wn": np.random.randn(D // 2, D).astype(np.float32) * 0.02,
        "bn": np.random.randn(D // 2).astype(np.float32) * 0.02,
        "We": np.random.randn(D // 2, D).astype(np.float32) * 0.02,
        "be": np.random.randn(D // 2).astype(np.float32) * 0.02,
        "Wl1": np.random.randn(D // 2, D).astype(np.float32) * 0.02,
        "bl1": np.random.randn(D // 2).astype(np.float32) * 0.02,
        "Wl2": np.random.randn(1, D // 2).astype(np.float32) * 0.02,
        "bl2": np.random.randn(1).astype(np.float32) * 0.02,
        "gh_wih": np.random.randn(3 * D, D).astype(np.float32) * 0.02,
        "gh_whh": np.random.randn(3 * D, D).astype(np.float32) * 0.02,
        "gh_bih": np.random.randn(3 * D).astype(np.float32) * 0.02,
        "gh_bhh": np.random.randn(3 * D).astype(np.float32) * 0.02,
        "gs_wih": np.random.randn(3 * D, D).astype(np.float32) * 0.02,
        "gs_whh": np.random.randn(3 * D, D).astype(np.float32) * 0.02,
        "gs_bih": np.random.randn(3 * D).astype(np.float32) * 0.02,
        "gs_bhh": np.random.randn(3 * D).astype(np.float32) * 0.02,
    }
    out = kernel(**ins)
    print("kernel ran, out shape", out.shape, out.dtype, float(np.abs(out).mean()))



# revision 22
# speedup vs baseline: 1.1886x; 1.1886x over previous
"""Trainium2 Bass kernel for nn_Graph_Enhance_model (GNN message passing).

Self-contained: hardcodes shapes B=4,F=32,H=8,O=16,D=2048, 8 cores.

Phase A (edge waves): data-parallel over the 128 (b,f) frames, 16/core.
  Step-1 wave exploits UM0's structure: its msg_n half is broadcast over h,
  so the wave is a K=1024 matmul plus a rank-64 PSUM update built from
  Q = msg_n @ Wcat[1024:] and the step-0 softmax weights.
Phases B/C (GRUs): tensor-parallel over the 2048 hidden units, 256/core;
  each core computes ALL 128 frames for its unit slice. M_sum, All_human
  and s1 are exchanged with small HBM AllGathers (8-core mesh, ~5-15us).
"""

import os
import sys

for _p in ("/opt/trn_rl_repo", "/opt/pypackages"):
    if _p not in sys.path and os.path.isdir(_p):
        sys.path.append(_p)

import numpy as np
import ml_dtypes

import concourse.bass as bass
import concourse.bacc as bacc
import concourse.tile as tile
import concourse.mybir as mybir
from concourse import bass_utils
from concourse.masks import make_identity

BF16 = mybir.dt.bfloat16
F32 = mybir.dt.float32
AF = mybir.ActivationFunctionType
ALU = mybir.AluOpType
AX = mybir.AxisListType

NB = ml_dtypes.bfloat16

B, F, H, O, D = 4, 32, 8, 16, 2048
NFRAMES = B * F          # 128
NCORES = 8
FPC = NFRAMES // NCORES  # 16 frames per core
ROWS = H * O             # 128 rows per frame
KC = D // 128            # 16 K-chunks
NQ = FPC // 4            # 4 quads of 4 frames
UPC = D // NCORES        # 256 units per core (TP slice)
GPC = 3 * UPC            # 768 gate columns per core
NR = NFRAMES * H         # 1024 human rows globally
RCN = NR // 128          # 8 row chunks

_CACHE = {}
RG = [list(range(NCORES))]


def _build_nc():
    nc = bacc.Bacc("TRN2", target_bir_lowering=False, debug=False, num_devices=NCORES)

    dt_in = {}

    def din(name, shape, dt):
        dt_in[name] = nc.dram_tensor(name, shape, dt, kind="ExternalInput")
        return dt_in[name]

    # per-core phase A
    e0t = din("e0t", [NQ, D, 512], BF16)
    ot = din("ot", [D, FPC * O], BF16)
    # replicated phase A consts
    wcat = din("wcat", [D, D], BF16)
    bl1td = din("bl1t", [128, 8], F32)
    betd = din("bet", [128, 8], F32)
    wnt = din("wnt", [D, D // 2], BF16)
    wnb = din("wnb", [1, D // 2], BF16)
    wl2 = din("wl2", [128, 8], BF16)
    scatd = din("scat2", [128, 2, 512], BF16)
    # phase B (TP slices + replicated transposed inputs)
    pmatd = din("pmat", [128, FPC], BF16)
    htfd = din("ht_full", [D, NR], BF16)
    hrmd = din("h_rm_s", [NR, UPC], F32)
    whid = din("whi_s", [D, GPC], BF16)
    whhd = din("whh_s", [D, GPC], BF16)
    bhid = din("bhi_s", [1, GPC], BF16)
    bhhd = din("bhh_s", [1, GPC], BF16)
    # phase C
    wsid = din("wsi_s", [D, GPC], BF16)
    wshd = din("wsh_s", [D, GPC], BF16)
    bsid = din("bsi_s", [1, GPC], BF16)
    bshd = din("bsh_s", [1, GPC], BF16)
    sc4td = din("sc4t", [D, NFRAMES], BF16)
    sftd = din("sft", [D, NFRAMES], BF16)
    sc4sd = din("sc4_s", [NFRAMES, UPC], F32)
    sfsd = din("sf_s", [NFRAMES, UPC], F32)
    outp = nc.dram_tensor("outp", [NFRAMES, UPC], F32, kind="ExternalOutput")

    from contextlib import ExitStack

    with tile.TileContext(nc) as tc, ExitStack() as ctx:
        glob = ctx.enter_context(tc.tile_pool(name="glob", bufs=1))
        dram = ctx.enter_context(tc.tile_pool(name="dram", bufs=1, space="DRAM"))

        # DRAM bounce buffers for collectives
        msum_cin = dram.tile([D, 128], BF16)
        msum_cout = dram.tile([NCORES * D, 128], BF16, addr_space="Shared")
        ah_cin = dram.tile([UPC, NFRAMES], BF16)
        ah_cout = dram.tile([D, NFRAMES], BF16, addr_space="Shared")
        s1_cin = dram.tile([UPC, NFRAMES], BF16)
        s1_cout = dram.tile([D, NFRAMES], BF16, addr_space="Shared")

        ones_b = glob.tile([1, 512], BF16)
        nc.vector.memset(ones_b, 1.0)
        ident128 = glob.tile([128, 128], BF16)
        make_identity(nc, ident128)

        wl2_sb = glob.tile([128, 8], BF16)
        nc.sync.dma_start(out=wl2_sb, in_=wl2.ap())
        bl1t_sb = glob.tile([128, 8], F32)
        nc.sync.dma_start(out=bl1t_sb, in_=bl1td.ap())
        bet_sb = glob.tile([128, 8], F32)
        nc.sync.dma_start(out=bet_sb, in_=betd.ap())
        scat_sb = glob.tile([128, 2, 512], BF16)
        nc.sync.dma_start(out=scat_sb, in_=scatd.ap())
        pmat_sb = glob.tile([128, FPC], BF16)
        nc.sync.dma_start(out=pmat_sb, in_=pmatd.ap())

        msgn_sb = glob.tile([128, 8, FPC * O], BF16)    # msg_n^T [1024, 256]
        msum_f = glob.tile([128, KC, 128], F32)         # M_sum^T local (sum over o)

        # phase B weights: resident whole kernel; prefetch on gpsimd queue
        whi_sb = glob.tile([128, KC, GPC], BF16)
        nc.gpsimd.dma_start(out=whi_sb, in_=whid.ap().rearrange("(kc p) m -> p kc m", p=128))
        whh_sb = glob.tile([128, KC, GPC], BF16)
        nc.gpsimd.dma_start(out=whh_sb, in_=whhd.ap().rearrange("(kc p) m -> p kc m", p=128))
        bhi_sb = glob.tile([1, GPC], BF16)
        nc.gpsimd.dma_start(out=bhi_sb, in_=bhid.ap())
        bhh_sb = glob.tile([1, GPC], BF16)
        nc.gpsimd.dma_start(out=bhh_sb, in_=bhhd.ap())

        with tc.tile_pool(name="paq", bufs=1) as paq:
            q_sb = paq.tile([128, 2, D], BF16)          # Q for quad-pairs

            with (
                tc.tile_pool(name="pwcat", bufs=1) as pwcat,
                tc.tile_pool(name="pa", bufs=1) as pa,
                tc.tile_pool(name="pa1", bufs=1) as pa1,
            ):
                wcat_sb = pwcat.tile([128, KC, D], BF16)
                nc.sync.dma_start(out=wcat_sb, in_=wcat.ap().rearrange("(kc p) m -> p kc m", p=128))

                # ---------------- Phase 0: msg_n^T = Wn @ O^T + bn ----------------
                with nc.named_scope("ph0"):
                    with (
                        tc.tile_pool(name="p0", bufs=1) as p0,
                        tc.tile_pool(name="p0ps", bufs=4, space="PSUM") as p0ps,
                    ):
                        wnb_sb = p0.tile([1, D // 2], BF16)
                        nc.sync.dma_start(out=wnb_sb, in_=wnb.ap())
                        ot_sb = p0.tile([128, KC, FPC * O], BF16)
                        nc.sync.dma_start(out=ot_sb, in_=ot.ap().rearrange("(kc p) n -> p kc n", p=128))
                        for quar in range(4):
                            wn_sb = p0.tile([128, KC, 256], BF16, tag="wn")
                            nc.sync.dma_start(out=wn_sb, in_=wnt.ap()[:, quar * 256:(quar + 1) * 256]
                                              .rearrange("(kc p) m -> p kc m", p=128))
                            for mt2 in range(2):
                                mt = quar * 2 + mt2
                                pm = p0ps.tile([128, FPC * O], F32, tag="pm")
                                for kc in range(KC):
                                    nc.tensor.matmul(pm, lhsT=wn_sb[:, kc, mt2 * 128:(mt2 + 1) * 128],
                                                     rhs=ot_sb[:, kc, :], start=(kc == 0), stop=False)
                                nc.tensor.matmul(pm, lhsT=wnb_sb[0:1, mt * 128:(mt + 1) * 128],
                                                 rhs=ones_b[0:1, 0:FPC * O], start=False, stop=True)
                                nc.scalar.copy(msgn_sb[:, mt, :], pm)

                # ---------------- Q = msg_n @ Wcat[1024:, :]  (for step-1 rank update) ----
                with nc.named_scope("phQ"):
                    with tc.tile_pool(name="pqps", bufs=2, space="PSUM") as pqps:
                        for qq in range(2):
                            for ms in range(4):
                                pqp = pqps.tile([128, 512], F32, tag="pqp")
                                for j in range(8):
                                    nc.tensor.matmul(pqp, lhsT=msgn_sb[:, j, qq * 128:(qq + 1) * 128],
                                                     rhs=wcat_sb[:, 8 + j, ms * 512:(ms + 1) * 512],
                                                     start=(j == 0), stop=(j == 7))
                                nc.scalar.copy(q_sb[:, qq, ms * 512:(ms + 1) * 512], pqp)

                # ---------------- Phase A: 2 propagation steps over edges ----------------
                with tc.tile_pool(name="paps", bufs=4, space="PSUM") as paps, \
                     tc.tile_pool(name="papss", bufs=2, space="PSUM") as papss:
                    for q in range(NQ):
                        xq = pa.tile([128, KC, 512], BF16, tag="xq")
                        nc.sync.dma_start(out=xq, in_=e0t.ap()[q].rearrange("(kc p) n -> p kc n", p=128))
                        um1t = pa1.tile([128, 8, 512], BF16, tag="um1t")
                        wscat = pa1.tile([128, 512], BF16, tag="wscat")
                        for step in range(2):
                            with nc.named_scope(f"q{q}s{step}"):
                                def chain(pt, mt):
                                    if step == 0:
                                        for kc in range(KC):
                                            nc.tensor.matmul(pt, lhsT=wcat_sb[:, kc, mt * 128:(mt + 1) * 128],
                                                             rhs=xq[:, kc, :], start=(kc == 0), stop=(kc == KC - 1))
                                    else:
                                        for kc in range(8):
                                            nc.tensor.matmul(pt, lhsT=wcat_sb[:, kc, mt * 128:(mt + 1) * 128],
                                                             rhs=um1t[:, kc, :], start=(kc == 0), stop=False)
                                        nc.tensor.matmul(pt, lhsT=q_sb[:, q // 2, mt * 128:(mt + 1) * 128],
                                                         rhs=wscat, start=False, stop=True)

                                # --- a-wave: relu(X @ Wl1^T + bl1), transposed ---
                                relu_sb = pa1.tile([128, 8, 512], BF16, tag="relu")
                                for mt in range(8, 16):
                                    pw_a = paps.tile([128, 512], F32, tag="wave")
                                    chain(pw_a, mt)
                                    nc.scalar.activation(relu_sb[:, mt - 8, :], pw_a, AF.Relu,
                                                         bias=bl1t_sb[:, mt - 8:mt - 7])
                                # --- logits + softmax over o (groups of 16) ---
                                pl = papss.tile([1, 512], F32, tag="pl")
                                for kc2 in range(8):
                                    nc.tensor.matmul(pl, lhsT=wl2_sb[:, kc2:kc2 + 1],
                                                     rhs=relu_sb[:, kc2, :], start=(kc2 == 0), stop=(kc2 == 7))
                                pl3 = pl.rearrange("o (g i) -> o g i", i=16)
                                mx = pa1.tile([1, 32], F32, tag="mx")
                                nc.vector.reduce_max(mx, pl3, axis=AX.X)
                                sub = pa1.tile([1, 512], F32, tag="sub")
                                nc.vector.tensor_tensor(sub.rearrange("o (g i) -> o g i", i=16), pl3,
                                                        mx.broadcast_to((1, 32, 16)), op=ALU.subtract)
                                nc.scalar.activation(sub, sub, AF.Exp)
                                ex3 = sub.rearrange("o (g i) -> o g i", i=16)
                                sm = pa1.tile([1, 32], F32, tag="sm")
                                nc.vector.reduce_sum(sm, ex3, axis=AX.X)
                                rs = pa1.tile([1, 32], F32, tag="rs")
                                nc.vector.reciprocal(rs, sm)
                                w_sb = pa1.tile([1, 512], BF16, tag="w")
                                nc.vector.tensor_tensor(w_sb.rearrange("o (g i) -> o g i", i=16), ex3,
                                                        rs.broadcast_to((1, 32, 16)), op=ALU.mult)
                                # --- msg_e wave; w-broadcast MM emitted after 2 groups ---
                                e_ps = []
                                wb_sb = pa1.tile([128, 512], F32, tag="wb")

                                def combine(mt, pe):
                                    if step == 0:
                                        nc.vector.scalar_tensor_tensor(
                                            out=um1t[:, mt, :], in0=pe, scalar=bet_sb[:, mt:mt + 1],
                                            in1=wb_sb, op0=ALU.add, op1=ALU.mult)
                                    else:
                                        tmp = pa1.tile([128, 512], F32, tag="um2")
                                        nc.vector.scalar_tensor_tensor(
                                            out=tmp, in0=pe, scalar=bet_sb[:, mt:mt + 1],
                                            in1=wb_sb, op0=ALU.add, op1=ALU.mult)
                                        nc.vector.reduce_sum(msum_f[:, mt, q * 32:(q + 1) * 32],
                                                             tmp.rearrange("p (f h o) -> p f h o", f=4, h=8),
                                                             axis=AX.X)

                                for mt in range(8):
                                    pe = paps.tile([128, 512], F32, tag="wave")
                                    chain(pe, mt)
                                    e_ps.append(pe)
                                    if mt == 1:
                                        # broadcast w along partitions via K=1 matmul (PE waits
                                        # here on softmax, hidden under the first 2 MM groups)
                                        pw_b = papss.tile([128, 512], F32, tag="pw")
                                        nc.tensor.matmul(pw_b, lhsT=ones_b[0:1, 0:128], rhs=w_sb,
                                                         start=True, stop=True)
                                        nc.scalar.copy(wb_sb, pw_b)
                                    if mt >= 1:
                                        for cmt in ([0, 1] if mt == 1 else [mt]):
                                            combine(cmt, e_ps[cmt])
                                if step == 0:
                                    # rank-update rhs for step 1: scatter w0 over (f,o) rows
                                    nc.vector.tensor_tensor(wscat, scat_sb[:, q % 2, :], wb_sb, op=ALU.mult)
                                else:
                                    # msg_n half of M_sum: w1-weighted msg_n summed over o
                                    wb4 = wb_sb.rearrange("p (f h o) -> p f h o", f=4, h=8)
                                    for j in range(8):
                                        base = msgn_sb[:, j, q * 64:(q + 1) * 64]
                                        mn_bc = bass.AP(tensor=base.tensor, offset=base.offset,
                                                        ap=[list(base.ap[0]), [16, 4], [0, 8], [1, 16]])
                                        tmp = pa1.tile([128, 512], F32, tag="um2")
                                        nc.vector.tensor_tensor(
                                            tmp.rearrange("p (f h o) -> p f h o", f=4, h=8),
                                            mn_bc, wb4, op=ALU.mult)
                                        nc.vector.reduce_sum(msum_f[:, 8 + j, q * 32:(q + 1) * 32],
                                                             tmp.rearrange("p (f h o) -> p f h o", f=4, h=8),
                                                             axis=AX.X)

        # ---------------- Phase B: human GRU, TP over units ----------------
        with (
            tc.tile_pool(name="pcw", bufs=1) as pcw,
            tc.tile_pool(name="pb", bufs=2) as pb,
            tc.tile_pool(name="pb1", bufs=1) as pb1,
        ):
            # M_sum AllGather first: its sync DMA + gpsimd AG go ahead of
            # everything else queued at the A->B boundary
            msum_b = pcw.tile([128, KC, 128], BF16)
            with nc.named_scope("msum_ag"):
                for kc in range(KC):
                    nc.vector.tensor_copy(msum_b[:, kc, :], msum_f[:, kc, :])
                nc.sync.dma_start(out=msum_cin.rearrange("(kc p) n -> p kc n", p=128), in_=msum_b)
                nc.gpsimd.collective_compute(
                    "AllGather", ALU.bypass, replica_groups=RG,
                    ins=[msum_cin.opt()], outs=[msum_cout.opt()])
            # H^T row-chunk 0 separately so phase B's first gh chain starts
            # without waiting for the full 4MB load
            ht_a0 = pcw.tile([128, KC, 128], BF16)
            nc.sync.dma_start(out=ht_a0, in_=htfd.ap()[:, 0:128]
                              .rearrange("(kc p) n -> p kc n", p=128))
            ht_rest = pcw.tile([128, KC, NR - 128], BF16)
            nc.sync.dma_start(out=ht_rest, in_=htfd.ap()[:, 128:NR]
                              .rearrange("(kc p) n -> p kc n", p=128))
            hrm_sb = pcw.tile([128, RCN, UPC], F32)
            nc.sync.dma_start(out=hrm_sb, in_=hrmd.ap().rearrange("(rc p) u -> p rc u", p=128))
            # phase C weights load during the msum AG window (gpsimd queue,
            # behind the AG instruction but well before phase C needs them)
            wsi_sb = pcw.tile([128, KC, GPC], BF16)
            nc.gpsimd.dma_start(out=wsi_sb, in_=wsid.ap().rearrange("(kc p) m -> p kc m", p=128))
            wsh_sb = pcw.tile([128, KC, GPC], BF16)
            nc.gpsimd.dma_start(out=wsh_sb, in_=wshd.ap().rearrange("(kc p) m -> p kc m", p=128))
            bsi_sb = pcw.tile([1, GPC], BF16)
            nc.gpsimd.dma_start(out=bsi_sb, in_=bsid.ap())
            bsh_sb = pcw.tile([1, GPC], BF16)
            nc.gpsimd.dma_start(out=bsh_sb, in_=bshd.ap())
            sc4t_sb = pcw.tile([128, KC, NFRAMES], BF16)
            nc.gpsimd.dma_start(out=sc4t_sb, in_=sc4td.ap().rearrange("(kc p) n -> p kc n", p=128))
            sft_sb = pcw.tile([128, KC, NFRAMES], BF16)
            nc.gpsimd.dma_start(out=sft_sb, in_=sftd.ap().rearrange("(kc p) n -> p kc n", p=128))
            sc4s_sb = pcw.tile([NFRAMES, UPC], F32)
            nc.gpsimd.dma_start(out=sc4s_sb, in_=sc4sd.ap())
            sfs_sb = pcw.tile([NFRAMES, UPC], F32)
            nc.gpsimd.dma_start(out=sfs_sb, in_=sfsd.ap())

            with (
                tc.tile_pool(name="pbps", bufs=2, space="PSUM") as pbps,
                tc.tile_pool(name="pbps2", bufs=1, space="PSUM") as pbps2,
            ):
              pah = pbps2.tile([128, 2, 128], F32)

              with nc.named_scope("phB"):
                for rc in range(RCN):
                    if rc == 0:
                        htc = ht_a0
                        rsl = slice(0, 128)
                    else:
                        htc = ht_rest
                        rsl = slice((rc - 1) * 128, rc * 128)
                    msT = pb.tile([128, KC, 128], BF16, tag="msT")
                    nc.sync.dma_start(out=msT, in_=msum_cout[rc * D:(rc + 1) * D, :]
                                      .rearrange("(kc p) n -> p kc n", p=128))
                    p_rz = pbps.tile([128, 512], F32, tag="prz")
                    for kc in range(KC):
                        nc.tensor.matmul(p_rz, lhsT=htc[:, kc, rsl], rhs=whh_sb[:, kc, 0:512],
                                         start=(kc == 0), stop=False)
                    p_hn = pbps.tile([128, 256], F32, tag="phn")
                    for kc in range(KC):
                        nc.tensor.matmul(p_hn, lhsT=htc[:, kc, rsl], rhs=whh_sb[:, kc, 512:768],
                                         start=(kc == 0), stop=False)
                    nc.tensor.matmul(p_hn, lhsT=ones_b[0:1, 0:128], rhs=bhh_sb[0:1, 512:768],
                                     start=False, stop=True)
                    for kc in range(KC):
                        nc.tensor.matmul(p_rz, lhsT=msT[:, kc, :], rhs=whi_sb[:, kc, 0:512],
                                         start=False, stop=False)
                    nc.tensor.matmul(p_rz, lhsT=ones_b[0:1, 0:128], rhs=bhh_sb[0:1, 0:512],
                                     start=False, stop=False)
                    nc.tensor.matmul(p_rz, lhsT=ones_b[0:1, 0:128], rhs=bhi_sb[0:1, 0:512],
                                     start=False, stop=True)
                    p_in = pbps.tile([128, 256], F32, tag="pin")
                    for kc in range(KC):
                        nc.tensor.matmul(p_in, lhsT=msT[:, kc, :], rhs=whi_sb[:, kc, 512:768],
                                         start=(kc == 0), stop=False)
                    nc.tensor.matmul(p_in, lhsT=ones_b[0:1, 0:128], rhs=bhi_sb[0:1, 512:768],
                                     start=False, stop=True)
                    # elementwise GRU combine -> humans (bf16) for this row chunk
                    rz = pb1.tile([128, 512], F32, tag="rz")
                    nc.scalar.activation(rz, p_rz, AF.Sigmoid)
                    t1 = pb1.tile([128, 256], F32, tag="t1")
                    nc.vector.tensor_tensor(t1, rz[:, 0:256], p_hn, op=ALU.mult)
                    t2 = pb1.tile([128, 256], F32, tag="t2")
                    nc.vector.tensor_tensor(t2, t1, p_in, op=ALU.add)
                    n_sb = pb1.tile([128, 256], F32, tag="n")
                    nc.scalar.activation(n_sb, t2, AF.Tanh)
                    t3 = pb1.tile([128, 256], F32, tag="t3")
                    nc.vector.tensor_tensor(t3, hrm_sb[:, rc, :], n_sb, op=ALU.subtract)
                    t4 = pb1.tile([128, 256], F32, tag="t4")
                    nc.vector.tensor_tensor(t4, rz[:, 256:512], t3, op=ALU.mult)
                    hum_bt = pb1.tile([128, 256], BF16, tag="hum")
                    nc.vector.tensor_tensor(hum_bt, n_sb, t4, op=ALU.add)
                    # All_human^T contribution: mean over h via pmat
                    for u2 in range(2):
                        nc.tensor.matmul(pah[:, u2, rc * FPC:(rc + 1) * FPC],
                                         lhsT=hum_bt[:, u2 * 128:(u2 + 1) * 128],
                                         rhs=pmat_sb, start=True, stop=True)

                ahT = pb1.tile([128, 2, 128], BF16, tag="ahT")
                nc.scalar.copy(ahT[:, 0, :], pah[:, 0, :])
                nc.scalar.copy(ahT[:, 1, :], pah[:, 1, :])

            with nc.named_scope("ah_ag"):
                nc.sync.dma_start(out=ah_cin.rearrange("(c p) n -> p c n", p=128), in_=ahT)
                nc.gpsimd.collective_compute(
                    "AllGather", ALU.bypass, replica_groups=RG,
                    ins=[ah_cin.opt()], outs=[ah_cout.opt()])

            # ---------------- Phase C: two S-node GRUs, TP over units ----------------
            with (
                tc.tile_pool(name="pc1", bufs=1) as pc1,
                tc.tile_pool(name="pcsm", bufs=1) as pcsm,
                tc.tile_pool(name="pcps", bufs=2, space="PSUM") as pcps,
                tc.tile_pool(name="pctps", bufs=2, space="PSUM") as pctps,
            ):
                def s_gru_gates(xt_sb, ht_sb, scope):
                    """gates for one S-GRU step: x @ wsi + h @ wsh + biases."""
                    with nc.named_scope(scope):
                        p_rz = pcps.tile([128, 512], F32, tag="srz")
                        for kc in range(KC):
                            nc.tensor.matmul(p_rz, lhsT=ht_sb[:, kc, :], rhs=wsh_sb[:, kc, 0:512],
                                             start=(kc == 0), stop=False)
                        for kc in range(KC):
                            nc.tensor.matmul(p_rz, lhsT=xt_sb[:, kc, :], rhs=wsi_sb[:, kc, 0:512],
                                             start=False, stop=False)
                        nc.tensor.matmul(p_rz, lhsT=ones_b[0:1, 0:128], rhs=bsh_sb[0:1, 0:512],
                                         start=False, stop=False)
                        nc.tensor.matmul(p_rz, lhsT=ones_b[0:1, 0:128], rhs=bsi_sb[0:1, 0:512],
                                         start=False, stop=True)
                        p_hn = pcps.tile([128, 256], F32, tag="shn")
                        for kc in range(KC):
                            nc.tensor.matmul(p_hn, lhsT=ht_sb[:, kc, :], rhs=wsh_sb[:, kc, 512:768],
                                             start=(kc == 0), stop=False)
                        nc.tensor.matmul(p_hn, lhsT=ones_b[0:1, 0:128], rhs=bsh_sb[0:1, 512:768],
                                         start=False, stop=True)
                        p_in = pcps.tile([128, 256], F32, tag="sin")
                        for kc in range(KC):
                            nc.tensor.matmul(p_in, lhsT=xt_sb[:, kc, :], rhs=wsi_sb[:, kc, 512:768],
                                             start=(kc == 0), stop=False)
                        nc.tensor.matmul(p_in, lhsT=ones_b[0:1, 0:128], rhs=bsi_sb[0:1, 512:768],
                                         start=False, stop=True)
                        return p_rz, p_hn, p_in

                def s_gru_elem(p_rz, p_hn, p_in, h_sb, out_sb):
                    rz = pcsm.tile([128, 512], F32, tag="crz")
                    nc.scalar.activation(rz, p_rz, AF.Sigmoid)
                    u1 = pcsm.tile([128, 256], F32, tag="u1")
                    nc.vector.tensor_tensor(u1, rz[:, 0:256], p_hn, op=ALU.mult)
                    u2 = pcsm.tile([128, 256], F32, tag="u2")
                    nc.vector.tensor_tensor(u2, u1, p_in, op=ALU.add)
                    n1 = pcsm.tile([128, 256], F32, tag="n1")
                    nc.scalar.activation(n1, u2, AF.Tanh)
                    u3 = pcsm.tile([128, 256], F32, tag="u3")
                    nc.vector.tensor_tensor(u3, h_sb, n1, op=ALU.subtract)
                    u4 = pcsm.tile([128, 256], F32, tag="u4")
                    nc.vector.tensor_tensor(u4, rz[:, 256:512], u3, op=ALU.mult)
                    nc.vector.tensor_tensor(out_sb, n1, u4, op=ALU.add)

                ah_all = pc1.tile([128, KC, 128], BF16)
                nc.sync.dma_start(out=ah_all, in_=ah_cout.rearrange("(kc p) n -> p kc n", p=128))
                p_rz1, p_hn1, p_in1 = s_gru_gates(ah_all, sc4t_sb, "phC1")
                s1_b = pc1.tile([NFRAMES, UPC], BF16)
                s_gru_elem(p_rz1, p_hn1, p_in1, sc4s_sb, s1_b)
                # transpose s1 slice -> [units, frames], gather to full s1^T
                s1T = pc1.tile([128, 2, 128], BF16)
                for u2 in range(2):
                    ptp = pctps.tile([128, 128], BF16, tag="tp")
                    nc.tensor.transpose(ptp, s1_b[:, u2 * 128:(u2 + 1) * 128], ident128)
                    nc.scalar.copy(s1T[:, u2, :], ptp)
                with nc.named_scope("s1_ag"):
                    nc.sync.dma_start(out=s1_cin.rearrange("(c p) n -> p c n", p=128), in_=s1T)
                    nc.gpsimd.collective_compute(
                        "AllGather", ALU.bypass, replica_groups=RG,
                        ins=[s1_cin.opt()], outs=[s1_cout.opt()])
                s1t_all = pc1.tile([128, KC, 128], BF16)
                nc.sync.dma_start(out=s1t_all, in_=s1_cout.rearrange("(kc p) n -> p kc n", p=128))
                p_rz2, p_hn2, p_in2 = s_gru_gates(s1t_all, sft_sb, "phC2")
                out_sb = pc1.tile([NFRAMES, UPC], F32)
                s_gru_elem(p_rz2, p_hn2, p_in2, sfs_sb, out_sb)
                nc.sync.dma_start(out=outp.ap(), in_=out_sb)

    nc.compile()
    return nc


def _prep_in_maps(inputs):
    E = np.ascontiguousarray(inputs["H_O_edges"].reshape(NFRAMES, ROWS, D))
    On = inputs["O_nodes"].reshape(NFRAMES, O, D)
    Hn = inputs["H_nodes"].reshape(NFRAMES, H, D)
    Sc4 = inputs["S_node_C4"].reshape(NFRAMES, D)
    Sf = np.ascontiguousarray(inputs["final_S_node"].transpose(0, 2, 1)).reshape(NFRAMES, D)

    whi_t = np.ascontiguousarray((inputs["gh_wih"] / float(O)).T)
    whh_t = np.ascontiguousarray(inputs["gh_whh"].T)
    wsi_t = np.ascontiguousarray(inputs["gs_wih"].T)
    wsh_t = np.ascontiguousarray(inputs["gs_whh"].T)

    def slice_gates(Wt, c):
        return np.ascontiguousarray(np.concatenate(
            [Wt[:, g * D + c * UPC:g * D + (c + 1) * UPC] for g in range(3)], axis=1))

    def slice_bias(b, c):
        return np.concatenate(
            [b[g * D + c * UPC:g * D + (c + 1) * UPC] for g in range(3)])[None, :]

    # scat2[:, par, :]: maps quad-column (f,h,o) to Q-pair row (f + 4*par, o)
    scat = np.zeros((128, 2, 512), np.float32)
    for par in range(2):
        for f in range(4):
            for h in range(H):
                for o in range(O):
                    scat[(f + 4 * par) * O + o, par, f * 128 + h * O + o] = 1.0

    shared = {
        "wcat": np.ascontiguousarray(
            np.concatenate([inputs["We"], inputs["Wl1"]], axis=0).T).astype(NB),
        "bl1t": np.ascontiguousarray(inputs["bl1"].reshape(8, 128).T).astype(np.float32),
        "bet": np.ascontiguousarray(inputs["be"].reshape(8, 128).T).astype(np.float32),
        "pmat": np.ascontiguousarray(np.kron(np.eye(FPC), np.ones((H, 1))) / H).astype(NB),
        "wnt": np.ascontiguousarray(inputs["Wn"].T).astype(NB),
        "wnb": inputs["bn"][None, :].astype(NB),
        "wl2": np.ascontiguousarray(inputs["Wl2"][0].reshape(8, 128).T).astype(NB),
        "scat2": scat.astype(NB),
        "ht_full": np.ascontiguousarray(Hn.reshape(NR, D).T).astype(NB),
        "sc4t": np.ascontiguousarray(Sc4.T).astype(NB),
        "sft": np.ascontiguousarray(Sf.T).astype(NB),
    }

    in_maps = []
    for c in range(NCORES):
        fr = slice(c * FPC, (c + 1) * FPC)
        us = slice(c * UPC, (c + 1) * UPC)
        Ec = E[fr]  # [16, 128, 2048]
        e0t = np.ascontiguousarray(
            Ec.reshape(NQ, 4, ROWS, D).transpose(0, 3, 1, 2).reshape(NQ, D, 512)).astype(NB)
        ot = np.ascontiguousarray(On[fr].reshape(FPC * O, D).T).astype(NB)
        m = dict(shared)
        m.update({
            "e0t": e0t,
            "ot": ot,
            "h_rm_s": np.ascontiguousarray(Hn.reshape(NR, D)[:, us]).astype(np.float32),
            "whi_s": slice_gates(whi_t, c).astype(NB),
            "whh_s": slice_gates(whh_t, c).astype(NB),
            "bhi_s": slice_bias(inputs["gh_bih"], c).astype(NB),
            "bhh_s": slice_bias(inputs["gh_bhh"], c).astype(NB),
            "wsi_s": slice_gates(wsi_t, c).astype(NB),
            "wsh_s": slice_gates(wsh_t, c).astype(NB),
            "bsi_s": slice_bias(inputs["gs_bih"], c).astype(NB),
            "bsh_s": slice_bias(inputs["gs_bhh"], c).astype(NB),
            "sc4_s": np.ascontiguousarray(Sc4[:, us]).astype(np.float32),
            "sf_s": np.ascontiguousarray(Sf[:, us]).astype(np.float32),
        })
        in_maps.append(m)
    return in_maps


LAST_RESULT = None


def kernel(**inputs):
    global LAST_RESULT
    if "nc" not in _CACHE:
        _CACHE["nc"] = _build_nc()
    nc = _CACHE["nc"]
    in_maps = _prep_in_maps(inputs)
    trace = os.environ.get("KERNEL_TRACE", "0") == "1"
    res = bass_utils.run_bass_kernel_spmd(
        nc, in_maps, core_ids=list(range(NCORES)), trace=trace)
    LAST_RESULT = res
    out = np.concatenate([res.results[c]["outp"] for c in range(NCORES)], axis=1)
    return np.ascontiguousarray(out.reshape(B, F, D)).astype(np.float32)


if __name__ == "__main__":
    np.random.seed(0)
    ins = {
        "S_node_C4": np.random.randn(B, F, D).astype(np.float32),
        "final_S_node": np.random.randn(B, D, F).astype(np.float32),
        "H_nodes": np.random.randn(B, F, H, D).astype(np.float32),
        "O_nodes": np.random.randn(B, F, O, D).astype(np.float32),
        "H_O_edges": np.random.randn(B, F, H, O, D).astype(np.float32),
        "Wn": np.random.randn(D // 2, D).astype(np.float32) * 0.02,
        "bn": np.random.randn(D // 2).astype(np.float32) * 0.02,
        "We": np.random.randn(D // 2, D).astype(np.float32) * 0.02,
        "be": np.random.randn(D // 2).astype(np.float32) * 0.02,
        "Wl1": np.random.randn(D // 2, D).astype(np.float32) * 0.02,
        "bl1": np.random.randn(D // 2).astype(np.float32) * 0.02,
        "Wl2": np.random.randn(1, D // 2).astype(np.float32) * 0.02,
        "bl2": np.random.randn(1).astype(np.float32) * 0.02,
        "gh_wih": np.random.randn(3 * D, D).astype(np.float32) * 0.02,
        "gh_whh": np.random.randn(3 * D, D).astype(np.float32) * 0.02,
        "gh_bih": np.random.randn(3 * D).astype(np.float32) * 0.02,
        "gh_bhh": np.random.randn(3 * D).astype(np.float32) * 0.02,
        "gs_wih": np.random.randn(3 * D, D).astype(np.float32) * 0.02,
        "gs_whh": np.random.randn(3 * D, D).astype(np.float32) * 0.02,
        "gs_bih": np.random.randn(3 * D).astype(np.float32) * 0.02,
        "gs_bhh": np.random.randn(3 * D).astype(np.float32) * 0.02,
    }
    out = kernel(**ins)
    print("kernel ran, out shape", out.shape, out.dtype, float(np.abs(out).mean()))
